# revision 2
# baseline (speedup 1.0000x reference)
"""Trainium2 Bass kernel for nn_CELoss_15745350107749 (calibration ECE/MCE).

Computes, for logits [260000, 1024] f32 and labels [260000] int:
  conf[r] = max softmax(logits[r])  (== 1 / sum_j exp(l_rj - max_j l_rj))
  acc[r]  = (argmax_j l_rj == labels[r])
then equal-mass bins the sorted confidences into 20 bins and returns
(ece, mce) over |sum(conf) - sum(acc)| / bin_size per bin.

Sharding: data-parallel over N across 8 NeuronCores. Each core streams its
[32500, 1024] f32 shard from HBM once (memory-bound pass):
  - DVE: segmented reduce_max (negate=True -> -max, used as exp bias)
  - ACT: exp(l - max) with accum_out -> per-row softmax denominator s
  - DVE: is_equal(-logits[r, label_r], -max_r) -> per-row accuracy
The host supplies -logits[r, label_r] (an O(N) gather), then does the global
equal-mass binning on the N-length conf/acc vectors (the [N, C] tensor never
leaves the cores).
"""

import sys

if "/opt/trn_rl_repo" not in sys.path:
    sys.path.insert(0, "/opt/trn_rl_repo")

import numpy as np

N = 260000
C = 1024
NCORES = 8
SHARD = N // NCORES  # 32500
P = 128  # SBUF partitions
RPP = 8  # rows per partition per chunk
RPC = P * RPP  # 1024 rows per chunk (4MB DMA)
N_BINS = 20

# Chunk bases: 31 aligned chunks + one tail chunk re-reading the final 1024
# rows (rows 31476..32499); the 268-row overlap recomputes identical values.
BASES = [c * RPC for c in range(SHARD // RPC)] + [SHARD - RPC]
NCH = len(BASES)  # 32
COLS = NCH * RPP  # 256

TRACE = False
TRACE_KW = {}
LAST_RESULTS = None


def _build_bass(reps=1, rpp=RPP, bufs=4, mode="full", name=None):
    """Build the per-core Bass module.

    reps: repeat the full streaming pass this many times inside the NEFF
          (timing aid; outputs are identical to reps=1).
    mode: "full" | "dma_only" (skip compute) | "compute_only" (single
          resident tile, no per-chunk DMA) -- ablations for bottleneck
          attribution.
    """
    from contextlib import ExitStack

    import concourse.tile as tile
    from concourse import bacc, mybir

    rpc = P * rpp
    bases = [c * rpc for c in range(SHARD // rpc)]
    if SHARD % rpc:
        bases.append(SHARD - rpc)
    nch = len(bases)
    cols = nch * rpp

    f32 = mybir.dt.float32
    if name is None:
        name = f"ce_calib_r{reps}_{mode}_rpp{rpp}_b{bufs}"
    nc = bacc.Bacc(None, target_bir_lowering=False, name=name)

    x = nc.dram_tensor("x", [SHARD, C], f32, kind="ExternalInput")
    gneg = nc.dram_tensor("gneg", [P, cols], f32, kind="ExternalInput")
    s_out = nc.dram_tensor("s_out", [P, cols], f32, kind="ExternalOutput")
    acc_out = nc.dram_tensor("acc_out", [P, cols], f32, kind="ExternalOutput")

    with tile.TileContext(nc) as tc, ExitStack() as ctx:
        xpool = ctx.enter_context(tc.tile_pool(name="xin", bufs=bufs))
        mpool = ctx.enter_context(tc.tile_pool(name="nm", bufs=bufs))
        epool = ctx.enter_context(tc.tile_pool(name="esc", bufs=2))
        stat = ctx.enter_context(tc.tile_pool(name="stat", bufs=1))

        gneg_sb = stat.tile([P, cols], f32, tag="gneg_sb")
        nc.sync.dma_start(out=gneg_sb[:], in_=gneg[:, :])
        s_stage = stat.tile([P, cols], f32, tag="s_stage")
        acc_stage = stat.tile([P, cols], f32, tag="acc_stage")

        xres = None
        if mode == "compute_only":
            xres = stat.tile([P, rpp, C], f32, tag="xres")
            nc.sync.dma_start(
                out=xres[:],
                in_=x[0:rpc, :].rearrange("(p s) c -> p s c", s=rpp),
            )

        for _ in range(reps):
            for ci, base in enumerate(bases):
                if mode == "compute_only":
                    xt = xres
                else:
                    xt = xpool.tile([P, rpp, C], f32, tag="xt")
                    src = x[base : base + rpc, :].rearrange(
                        "(p s) c -> p s c", s=rpp
                    )
                    nc.sync.dma_start(out=xt[:], in_=src)

                if mode == "dma_only":
                    continue

                # nm[p, s] = -max_c x[base + p*rpp + s, c]
                nm = mpool.tile([P, rpp], f32, tag="nm")
                nc.vector.tensor_reduce(
                    out=nm[:],
                    in_=xt[:],
                    axis=mybir.AxisListType.X,
                    op=mybir.AluOpType.max,
                    negate=True,
                )

                # s_stage[p, col] = sum_c exp(x[row, c] - max_row)
                for s in range(rpp):
                    col = ci * rpp + s
                    et = epool.tile([P, C], f32, tag="et")
                    nc.scalar.activation(
                        out=et[:],
                        in_=xt[:, s, :],
                        func=mybir.ActivationFunctionType.Exp,
                        bias=nm[:, s : s + 1],
                        scale=1.0,
                        accum_out=s_stage[:, col : col + 1],
                    )

                # acc = (logits[row, label_row] == max_row), via negated operands
                nc.vector.tensor_tensor(
                    out=acc_stage[:, ci * rpp : (ci + 1) * rpp],
                    in0=gneg_sb[:, ci * rpp : (ci + 1) * rpp],
                    in1=nm[:],
                    op=mybir.AluOpType.is_equal,
                )

        if mode == "dma_only":
            # Touch something cheap so outputs are defined.
            nc.vector.memset(s_stage[:], 0.0)
            nc.vector.memset(acc_stage[:], 0.0)

        nc.sync.dma_start(out=s_out[:, :], in_=s_stage[:])
        nc.sync.dma_start(out=acc_out[:, :], in_=acc_stage[:])

    nc.compile()
    return nc


def _ensure_axon_hook_stub():
    """run_bass_kernel_spmd's trace path imports antenv.axon_hooks, which is
    absent in some axon containers. Stub it so trace requests degrade to an
    untraced run instead of crashing. No-op when the real module exists or
    when running natively (the import never fires outside axon)."""
    try:
        import antenv.axon_hooks  # noqa: F401
    except Exception:
        import types

        m = types.ModuleType("antenv.axon_hooks")
        m.get_axon_ntff_profile_hook = lambda: None
        sys.modules["antenv.axon_hooks"] = m


def kernel(logits, labels):
    global LAST_RESULTS
    from concourse.bass_utils import run_bass_kernel_spmd

    _ensure_axon_hook_stub()

    logits = np.asarray(logits)
    assert logits.dtype == np.float32 and logits.shape == (N, C)
    labels_i = np.asarray(labels).astype(np.int64)

    nc = _build_bass()

    in_maps = []
    for k in range(NCORES):
        sh = logits[k * SHARD : (k + 1) * SHARD]
        lb = labels_i[k * SHARD : (k + 1) * SHARD]
        g = sh[np.arange(SHARD), lb]  # logits[r, label_r], O(N) gather
        gneg2d = np.empty((P, COLS), np.float32)
        for ci, base in enumerate(BASES):
            gneg2d[:, ci * RPP : (ci + 1) * RPP] = -g[base : base + RPC].reshape(
                P, RPP
            )
        in_maps.append({"x": np.ascontiguousarray(sh), "gneg": gneg2d})

    res = run_bass_kernel_spmd(
        nc, in_maps, core_ids=list(range(NCORES)), trace=TRACE, **TRACE_KW
    )
    LAST_RESULTS = res

    conf_all = np.empty(N, np.float32)
    acc_all = np.empty(N, np.float32)
    for k, r in enumerate(res.results):
        s2, a2 = r["s_out"], r["acc_out"]
        s_rows = np.empty(SHARD, np.float32)
        a_rows = np.empty(SHARD, np.float32)
        for ci, base in enumerate(BASES):
            s_rows[base : base + RPC] = s2[:, ci * RPP : (ci + 1) * RPP].reshape(RPC)
            a_rows[base : base + RPC] = a2[:, ci * RPP : (ci + 1) * RPP].reshape(RPC)
        conf_all[k * SHARD : (k + 1) * SHARD] = np.float32(1.0) / s_rows
        acc_all[k * SHARD : (k + 1) * SHARD] = a_rows

    # Global equal-mass binning (matches reference's stable argsort + reshape).
    order = np.argsort(conf_all, kind="stable")
    bin_size = N // N_BINS
    s_conf = conf_all[order].reshape(N_BINS, bin_size).astype(np.float64).sum(axis=1)
    s_acc = acc_all[order].reshape(N_BINS, bin_size).astype(np.float64).sum(axis=1)
    ce = np.abs(s_conf - s_acc) / bin_size
    return (np.float32(ce.mean()), np.float32(ce.max()))
